# revision 1
# baseline (speedup 1.0000x reference)
"""Trainium2 Bass kernel: AdaptiveAttentionFusion, pure data-parallel on 8 NeuronCores.

Strategy:
  - Shard batch B=16384 across 8 cores (2048 rows each); weights replicated.
  - Host-side weight folding: input projections absorbed into QKV / MLP weights
    (q_f = frontier @ (Wf@Wq) + ...), so raw inputs feed all matmuls directly.
  - All matmul operands fp16 (validated 2.9e-4 rel err), fp32 PSUM accumulation.
  - Natural layout (rows on partitions); lhsT = transposed activations via PE
    transposes packed 4-per-PSUM-bank; biases folded in via K=1 ones-row matmuls.
  - Single ACT table set (natural_log_exp): softmax exp; LN rstd = exp(-0.5*ln(var+eps));
    sigmoid = exp(-ln(1+exp(-x))).
"""
import os
import numpy as np

import concourse.bacc as bacc
import concourse.bass as bass
import concourse.tile as tile
from concourse import mybir
from concourse.bass_utils import run_bass_kernel_spmd
from concourse.masks import make_identity

D, H, KD = 512, 4, 128
LAG = 2
NCORES = 8
EPS = 1e-6
P = 128
F16 = mybir.dt.float16
F32 = mybir.dt.float32
AF = mybir.ActivationFunctionType
OP = mybir.AluOpType
AX = mybir.AxisListType

LAST_EXEC_TIME_NS = None
LAST_RESULTS = None


def ts(i, n=P):
    return slice(i * n, (i + 1) * n)


def _fold(inp):
    """Fold input projections into downstream weights. Returns (device arrays, ba2, affine)."""
    Ws = [inp['Wf'], inp['Wr'], inp['Wm']]
    bs = [inp['bf'], inp['br'], inp['bm']]
    Wqkv = np.concatenate([inp['Wq'].reshape(D, D), inp['Wk'].reshape(D, D),
                           inp['Wv'].reshape(D, D)], axis=1)          # [512, 1536]
    bqkv = np.concatenate([inp['bq'].reshape(-1), inp['bk'].reshape(-1),
                           inp['bv'].reshape(-1)])                    # [1536]
    f = {}
    f['wproj'] = np.concatenate(Ws, axis=1).astype(np.float16)        # [512, 1536]
    f['bproj'] = np.concatenate(bs)[None, :].astype(np.float16)       # [1, 1536]
    f['wqkv'] = np.concatenate([Ws[s] @ Wqkv for s in range(3)], axis=1).astype(np.float16)  # [512, 4608]
    f['bqkv'] = np.concatenate([bs[s] @ Wqkv + bqkv for s in range(3)])[None, :].astype(np.float16)
    A = [Ws[s] @ inp['Wa1'][s * D:(s + 1) * D] for s in range(3)]
    f['wh'] = np.concatenate(A, axis=1).astype(np.float16)            # [512, 768]
    f['bh'] = (inp['ba1'] + sum(bs[s] @ inp['Wa1'][s * D:(s + 1) * D]
                                for s in range(3)))[None, :].astype(np.float16)
    f['wo'] = inp['Wo'].reshape(D, D).astype(np.float16)
    f['bo'] = inp['bo'][None, :].astype(np.float16)
    f['wg'] = inp['Wg'].astype(np.float16)
    f['bg'] = inp['bg'][None, :].astype(np.float16)
    f['wa2t'] = inp['Wa2'].astype(np.float16)  # [256, 3]
    f['ba2'] = inp['ba2'][None, :].astype(np.float16)
    aff = np.stack([inp['gamma1'], inp['beta1'], inp['gamma2'], inp['beta2']]).astype(np.float32)
    return f, aff


def _pin_act_table(nc):
    """Instance-level override of insert_act_table_loads: make
    natural_log_exp_and_others the only selectable ACT table set, so the
    kernel never thrashes table loads (it covers exp/ln/relu/copy/identity).
    Set ids stay aligned with act_info.json ordering."""
    import types
    import bass_rust as _bass_rust
    from concourse.hw_specs import get_activation_tables

    def patched(self):
        has_activation = any(
            isinstance(i, mybir.InstActivation)
            for b in self.main_func.blocks
            for i in b.instructions
        )
        if not has_activation:
            return
        tables = [
            (name, fns if name == "natural_log_exp_and_others" else set())
            for name, fns in get_activation_tables(self.m.arch).items()
        ]
        _bass_rust.insert_act_table_loads(self, tables)

    nc.insert_act_table_loads = types.MethodType(patched, nc)


def _build(R, need_aff1, need_aff2):
    ntiles = R // P
    nc = bacc.Bacc()
    _pin_act_table(nc)

    x_ext = [nc.declare_dram_parameter(n, [R, D], F32, isOutput=False)
             for n in ("frontier", "cross_robot", "map_feat")]
    wproj_d = nc.declare_dram_parameter("wproj", [D, 3 * D], F16, isOutput=False)
    bproj_d = nc.declare_dram_parameter("bproj", [1, 3 * D], F16, isOutput=False)
    wqkv_d = nc.declare_dram_parameter("wqkv", [D, 9 * D], F16, isOutput=False)
    bqkv_d = nc.declare_dram_parameter("bqkv", [1, 9 * D], F16, isOutput=False)
    wh_d = nc.declare_dram_parameter("wh", [D, 768], F16, isOutput=False)
    bh_d = nc.declare_dram_parameter("bh", [1, 256], F16, isOutput=False)
    wo_d = nc.declare_dram_parameter("wo", [D, D], F16, isOutput=False)
    bo_d = nc.declare_dram_parameter("bo", [1, D], F16, isOutput=False)
    wg_d = nc.declare_dram_parameter("wg", [D, D], F16, isOutput=False)
    bg_d = nc.declare_dram_parameter("bg", [1, D], F16, isOutput=False)
    bxo_d = nc.declare_dram_parameter("bxo", [1, 2 * D], F16, isOutput=False)
    wa2t_d = nc.declare_dram_parameter("wa2t", [256, 3], F16, isOutput=False)
    ba2_d = nc.declare_dram_parameter("ba2", [1, 3], F16, isOutput=False)
    aff_d = None
    if need_aff1 or need_aff2:
        aff_d = nc.declare_dram_parameter("aff", [4, D], F32, isOutput=False)
    out_ext = nc.declare_dram_parameter("out", [R, D], F32, isOutput=True)

    def bcast(ap, parts=P):
        """Partition-broadcast DMA source AP (stride-0 partition dim)."""
        return bass.AP(tensor=ap.tensor, offset=ap.offset, ap=[[0, parts]] + list(ap.ap))

    with tile.TileContext(nc) as tc:
        import contextlib
        with contextlib.ExitStack() as ctx:
            const = ctx.enter_context(tc.tile_pool(name="const", bufs=1))
            p_raw = ctx.enter_context(tc.tile_pool(name="p_raw", bufs=3))
            p_rt = ctx.enter_context(tc.tile_pool(name="p_rt", bufs=4))
            p_x = ctx.enter_context(tc.tile_pool(name="p_x", bufs=4))
            p_qkv = ctx.enter_context(tc.tile_pool(name="p_qkv", bufs=2))
            p_ctx = ctx.enter_context(tc.tile_pool(name="p_ctx", bufs=3))
            p_y = ctx.enter_context(tc.tile_pool(name="p_y", bufs=2))
            p_w = ctx.enter_context(tc.tile_pool(name="p_w", bufs=2))
            p_sm = ctx.enter_context(tc.tile_pool(name="p_sm", bufs=2))
            ps_t16 = ctx.enter_context(tc.tile_pool(name="ps_t16", bufs=3, space="PSUM"))
            ps_mm = ctx.enter_context(tc.tile_pool(name="ps_mm", bufs=4, space="PSUM"))

            # ---- constants ----
            wproj_sb = const.tile([P, 4, 3 * D], F16)
            nc.sync.dma_start(out=wproj_sb, in_=wproj_d[:, :].rearrange("(c p) f -> p c f", p=P))
            bproj_sb = const.tile([1, 3 * D], F16)
            nc.sync.dma_start(out=bproj_sb, in_=bproj_d[:, :])
            wqkv_sb = const.tile([P, 4, 9 * D], F16)
            for c in range(4):
                nc.sync.dma_start(out=wqkv_sb[:, c, :], in_=wqkv_d[c * P:(c + 1) * P, :])
            bqkv_sb = const.tile([1, 9 * D], F16)
            nc.sync.dma_start(out=bqkv_sb, in_=bqkv_d[:, :])
            wh_sb = const.tile([P, 4, 768], F16)
            nc.sync.dma_start(out=wh_sb, in_=wh_d[:, :].rearrange("(c p) f -> p c f", p=P))
            bh_sb = const.tile([1, 256], F16)
            nc.sync.dma_start(out=bh_sb, in_=bh_d[:, :])
            wo_sb = const.tile([P, 4, D], F16)
            nc.sync.dma_start(out=wo_sb, in_=wo_d[:, :].rearrange("(c p) f -> p c f", p=P))
            bo_sb = const.tile([1, D], F16)
            nc.sync.dma_start(out=bo_sb, in_=bo_d[:, :])
            wg_sb = const.tile([P, 4, D], F16)
            nc.sync.dma_start(out=wg_sb, in_=wg_d[:, :].rearrange("(c p) f -> p c f", p=P))
            bg_sb = const.tile([1, D], F16)
            nc.sync.dma_start(out=bg_sb, in_=bg_d[:, :])
            bxo_sb = const.tile([1, 2 * D], F16)
            nc.sync.dma_start(out=bxo_sb, in_=bxo_d[:, :])
            wa2_sb = const.tile([P, 2, 3], F16)
            nc.sync.dma_start(out=wa2_sb, in_=wa2t_d[:, :].rearrange("(c p) j -> p c j", p=P))
            ba2_sb = const.tile([1, 3], F16)
            nc.sync.dma_start(out=ba2_sb, in_=ba2_d[:, :])
            aff_sb = None
            if aff_d is not None:
                aff_sb = const.tile([P, 4, D], F32)
                nc.sync.dma_start(out=aff_sb, in_=bcast(aff_d[:, :]))

            ident16 = const.tile([P, P], F16)
            make_identity(nc, ident16)
            ones16 = const.tile([1, P], F16)
            nc.vector.memset(ones16, 1.0)
            eps_sb = const.tile([P, 1], F32)
            nc.vector.memset(eps_sb, EPS)

            isk = float(1.0 / np.sqrt(KD))

            def ln_stats(zin_ap):
                """LN stats: bn_stats/aggr on DVE, rsqrt via ln/exp on ScalarE."""
                st6 = p_sm.tile([P, 6], F32, name="st6", tag="st6", bufs=4)
                nc.vector.bn_stats(out=st6[:], in_=zin_ap)
                mv = p_sm.tile([P, 2], F32, name="mv", tag="mv", bufs=4)
                nc.vector.bn_aggr(out=mv[:], in_=st6[:])
                lnt = p_sm.tile([P, 1], F32, name="lnt", tag="lnt", bufs=4)
                nc.scalar.activation(out=lnt[:], in_=mv[:, 1:2], func=AF.Ln,
                                     bias=eps_sb[:], scale=1.0)
                rstd = p_sm.tile([P, 1], F32, name="rstd", tag="rstd", bufs=4)
                nc.scalar.activation(out=rstd[:], in_=lnt[:], func=AF.Exp, scale=-0.5)
                nmr = p_sm.tile([P, 1], F32, name="nmr", tag="nmr", bufs=4)
                nc.vector.scalar_tensor_tensor(out=nmr[:], in0=mv[:, 0:1], scalar=-1.0,
                                               in1=rstd[:], op0=OP.mult, op1=OP.mult)
                return rstd, nmr

            def ln_apply(zin_ap, zout, rstd, nmr, aff_idx):
                """zout = zin*rstd + nmr (ScalarE), then optional gamma/beta."""
                nc.scalar.activation(out=zout[:], in_=zin_ap, func=AF.Identity,
                                     scale=rstd[:], bias=nmr[:])
                if aff_idx is not None:
                    nc.vector.tensor_mul(out=zout[:], in0=zout[:], in1=aff_sb[:, aff_idx, :])
                    nc.vector.tensor_add(out=zout[:], in0=zout[:], in1=aff_sb[:, aff_idx + 1, :])

            def emitA1(i):
                """Matmul-heavy front: loads, transposes, projections, qkv, MLP,
                score products + reduces."""
                r0 = i * P
                # 1. load raw inputs
                raw = [p_raw.tile([P, D], F32, name=f"raw{s}", tag=f"raw{s}") for s in range(3)]
                for s in range(3):
                    nc.sync.dma_start(out=raw[s][:], in_=x_ext[s][r0:r0 + P, :])

                # 2. rawT (fp16): DVE cast, packed PE f16 transposes, ACT evict
                rawT = []
                for s in range(3):
                    r16 = p_raw.tile([P, D], F16, name=f"r16_{s}", tag=f"r16_{s}")
                    nc.vector.tensor_copy(r16[:], raw[s][:])
                    tp = ps_t16.tile([P, D], F16, name="tp16", tag="tp16")
                    for c in range(4):
                        nc.tensor.matmul(tp[:, ts(c)], lhsT=r16[:, ts(c)], rhs=ident16[:],
                                         is_transpose=True, start=(c == 0), stop=(c == 3))
                    rt = p_rt.tile([P, D], F16, name=f"rawT{s}", tag=f"rawT{s}")
                    nc.scalar.copy(out=rt[:], in_=tp[:])
                    rawT.append(rt)

                # 3. map-feat projection only (frontier/cross fold into o-psum in B)
                ps = ps_mm.tile([P, D], F32, name="mm", tag="mm")
                for c in range(4):
                    nc.tensor.matmul(ps[:], lhsT=rawT[2][:, ts(c)],
                                     rhs=wproj_sb[:, c, ts(2, D)],
                                     start=(c == 0), stop=False)
                nc.tensor.matmul(ps[:], lhsT=ones16[0:1, :], rhs=bproj_sb[0:1, ts(2, D)],
                                 start=False, stop=True)
                x_m = p_x.tile([P, D], F32, name="xm", tag="xm")
                nc.scalar.copy(out=x_m[:], in_=ps[:])

                # 4. qkv (fp16, bias folded): q,k into one tile (A1-local), v separate
                qk_t, v_t = [], []
                for s in range(3):
                    qk = p_qkv.tile([P, 2 * D], F16, name=f"qk{s}", tag=f"qk{s}", bufs=3)
                    vv = p_qkv.tile([P, D], F16, name=f"v{s}", tag=f"v{s}", bufs=3)
                    for g in range(3):
                        ps = ps_mm.tile([P, D], F32, name="mm", tag="mm")
                        col = s * 3 * D + g * D
                        for c in range(4):
                            nc.tensor.matmul(ps[:], lhsT=rawT[s][:, ts(c)],
                                             rhs=wqkv_sb[:, c, col:col + D],
                                             start=(c == 0), stop=False)
                        nc.tensor.matmul(ps[:], lhsT=ones16[0:1, :],
                                         rhs=bqkv_sb[0:1, col:col + D],
                                         start=False, stop=True)
                        dstt = qk[:, ts(g, D)] if g < 2 else vv[:]
                        nc.scalar.copy(out=dstt, in_=ps[:])
                    qk_t.append(qk)
                    v_t.append(vv)

                # 5. MLP hidden, TRANSPOSED: hT = relu(sum_s A_s^T @ raw_s^T + bh)
                ps_h = ps_mm.tile([P, D], F32, name="mm", tag="mm")
                for m in range(2):
                    for s in range(3):
                        for c in range(4):
                            nc.tensor.matmul(ps_h[:, ts(m)],
                                             lhsT=wh_sb[:, c, s * 256 + m * P: s * 256 + (m + 1) * P],
                                             rhs=rawT[s][:, ts(c)],
                                             start=(m == 0 and s == 0 and c == 0), stop=False)
                    nc.tensor.matmul(ps_h[:, ts(m)], lhsT=bh_sb[0:1, ts(m)], rhs=ones16[0:1, :],
                                     start=False, stop=(m == 1))
                hT = p_sm.tile([P, 2, P], F16, name="hT", tag="hT")
                nc.scalar.activation(out=hT[:], in_=ps_h[:, 0:256].rearrange("p (m r) -> p m r", m=2),
                                     func=AF.Relu)

                # 6. adaptive-weight logits awl = hT^T @ Wa2 + ba2 (PE)
                ps_a = ps_mm.tile([P, D], F32, name="mm", tag="mm")
                for m in range(2):
                    nc.tensor.matmul(ps_a[:, 0:3], lhsT=hT[:, m, :], rhs=wa2_sb[:, m, :],
                                     start=(m == 0), stop=False)
                nc.tensor.matmul(ps_a[:, 0:3], lhsT=ones16[0:1, :], rhs=ba2_sb[0:1, :],
                                 start=False, stop=True)
                awl = p_sm.tile([P, 3], F32, name="awl", tag="awl", bufs=3)
                nc.scalar.copy(out=awl[:], in_=ps_a[:, 0:3])

                # 7. attention scores: q*k product then per-head reduce (DVE)
                sc_t = p_sm.tile([P, 36], F32, name="sc", tag="sc", bufs=3)
                scr2 = p_sm.tile([P, 2, D], F16, name="scr2", tag="scr2", bufs=2)
                for j0 in range(0, 9, 2):
                    npair = min(2, 9 - j0)
                    for dj in range(npair):
                        j = j0 + dj
                        qi, si = divmod(j, 3)
                        nc.vector.tensor_mul(
                            out=scr2[:, dj, :],
                            in0=qk_t[qi][:, 0 * D:1 * D],
                            in1=qk_t[si][:, 1 * D:2 * D])
                    nc.vector.tensor_reduce(
                        out=sc_t[:, j0 * 4: (j0 + npair) * 4],
                        in_=scr2[:, 0:npair, :].rearrange("p a (h k) -> p (a h) k", h=H),
                        axis=AX.X, op=OP.add)

                return dict(r0=r0, x_m=x_m, rawT=rawT, v_t=v_t, sc_t=sc_t, awl=awl)

            def emitA2(st):
                """Softmax + ctx accumulation + adaptive-weight softmax."""
                sc_t, awl, v_t = st["sc_t"], st["awl"], st["v_t"]
                # 8a. adaptive weights aw = softmax(awl)
                aw_e = p_sm.tile([P, 3], F32, name="awe", tag="awe")
                aw_sum = p_sm.tile([P, 1], F32, name="aws", tag="aws")
                nc.scalar.activation(out=aw_e[:], in_=awl[:], func=AF.Exp,
                                     accum_out=aw_sum[:])
                aw_r = p_sm.tile([P, 1], F32, name="awr", tag="awr")
                nc.vector.reciprocal(out=aw_r[:], in_=aw_sum[:])
                aw_t = p_sm.tile([P, 3], F32, name="aw", tag="aw", bufs=4)
                nc.vector.tensor_scalar_mul(out=aw_t[:], in0=aw_e[:], scalar1=aw_r[:])

                # 8b. softmax over s
                e_t = p_sm.tile([P, 36], F32, name="e", tag="e")
                nc.scalar.activation(out=e_t[:], in_=sc_t[:], func=AF.Exp, scale=isk)
                e4 = e_t[:].rearrange("p (q s h) -> p q s h", q=3, s=3)
                ssum = p_sm.tile([P, 12], F32, name="ssum", tag="ssum")
                ss4 = ssum[:].rearrange("p (q h) -> p q h", q=3)
                nc.vector.tensor_add(out=ss4, in0=e4[:, :, 0, :], in1=e4[:, :, 1, :])
                nc.vector.tensor_add(out=ss4, in0=ss4, in1=e4[:, :, 2, :])
                rinv = p_sm.tile([P, 12], F32, name="rinv", tag="rinv")
                nc.vector.reciprocal(out=rinv[:], in_=ssum[:])
                attn = p_sm.tile([P, 36], F32, name="attn", tag="attn")
                a4 = attn[:].rearrange("p (q s h) -> p q s h", q=3, s=3)
                rb = rinv[:].rearrange("p (q h) -> p q h", q=3).unsqueeze(2).broadcast_to([P, 3, 3, H])
                nc.vector.tensor_mul(out=a4, in0=e4, in1=rb)

                # 9. ctx_q = sum_s bcast(attn[q,s,:]) * v_s  (stride-0 broadcast muls)
                ctx_t = [p_ctx.tile([P, D], F16, name=f"ctx{q}", tag=f"ctx{q}") for q in range(3)]
                ctmp = p_sm.tile([P, D], F16, name="ctmp", tag="ctmp", bufs=3)
                for qi in range(3):
                    for si in range(3):
                        ab = a4[:, qi, si, :].unsqueeze(-1).broadcast_to([P, H, KD])
                        vsl = v_t[si][:].rearrange("p (h k) -> p h k", h=H)
                        if si == 0:
                            nc.vector.tensor_mul(
                                out=ctx_t[qi][:].rearrange("p (h k) -> p h k", h=H),
                                in0=vsl, in1=ab)
                        else:
                            nc.vector.tensor_mul(
                                out=ctmp[:].rearrange("p (h k) -> p h k", h=H),
                                in0=vsl, in1=ab)
                            nc.vector.tensor_add(out=ctx_t[qi][:], in0=ctx_t[qi][:],
                                                 in1=ctmp[:])

                # ctx^T via packed PE f16 transposes (PE picks these up right
                # after ctx lands, ahead of phase B)
                ctxT = []
                for qi in range(3):
                    tp16 = ps_t16.tile([P, D], F16, name="tp16", tag="tp16")
                    for c in range(4):
                        nc.tensor.matmul(tp16[:, ts(c)], lhsT=ctx_t[qi][:, ts(c)], rhs=ident16[:],
                                         is_transpose=True, start=(c == 0), stop=(c == 3))
                    ct = p_ctx.tile([P, D], F16, name=f"ctxT{qi}", tag=f"ctxT{qi}")
                    nc.scalar.copy(out=ct[:], in_=tp16[:])
                    ctxT.append(ct)

                st["ctx_t"] = ctx_t
                st["aw_t"] = aw_t
                st["ctxT"] = ctxT
                return st

            def emitB(st):
                """Post-attention: ctx^T, o-proj(+frontier/cross proj residual in
                PSUM), LN1, weighted, gate, LN2, store."""
                r0, x_m, ctx_t, aw_t, rawT = (st["r0"], st["x_m"], st["ctx_t"],
                                              st["aw_t"], st["rawT"])
                ctxT = st["ctxT"]
                # 11. o-proj (+ x residual folded into PSUM for q=0,1) + LN1
                y_t = []
                for qi in range(3):
                    ps = ps_mm.tile([P, D], F32, name="mm", tag="mm")
                    for c in range(4):
                        nc.tensor.matmul(ps[:], lhsT=ctxT[qi][:, ts(c)], rhs=wo_sb[:, c, :],
                                         start=(c == 0), stop=False)
                    if qi < 2:
                        # accumulate x_qi = raw_qi @ W_qi directly into the same bank
                        for c in range(4):
                            nc.tensor.matmul(ps[:], lhsT=rawT[qi][:, ts(c)],
                                             rhs=wproj_sb[:, c, ts(qi, D)],
                                             start=False, stop=False)
                        nc.tensor.matmul(ps[:], lhsT=ones16[0:1, :], rhs=bxo_sb[0:1, ts(qi, D)],
                                         start=False, stop=True)
                        yin = ps[:]
                    else:
                        nc.tensor.matmul(ps[:], lhsT=ones16[0:1, :], rhs=bo_sb[0:1, :],
                                         start=False, stop=True)
                        yraw = p_y.tile([P, D], F32, name="yraw2", tag="yraw2")
                        nc.vector.tensor_add(out=yraw[:], in0=ps[:], in1=x_m[:])
                        yin = yraw[:]
                    rstd, nmr = ln_stats(yin)
                    yq = p_y.tile([P, D], F16, name=f"y{qi}", tag=f"y{qi}")
                    ln_apply(yin, yq, rstd, nmr, 0 if need_aff1 else None)
                    y_t.append(yq)

                # 12. weighted = sum_q aw_q * y_q
                w_t = p_w.tile([P, D], F16, name="w", tag="w")
                nc.vector.tensor_scalar_mul(out=w_t[:], in0=y_t[0][:], scalar1=aw_t[:, 0:1])
                for qi in (1, 2):
                    nc.vector.scalar_tensor_tensor(out=w_t[:], in0=y_t[qi][:],
                                                   scalar=aw_t[:, qi:qi + 1], in1=w_t[:],
                                                   op0=OP.mult, op1=OP.add)

                # 13. weighted^T: packed PE transpose (w already fp16)
                tpw = ps_t16.tile([P, D], F16, name="tp16", tag="tp16")
                for c in range(4):
                    nc.tensor.matmul(tpw[:, ts(c)], lhsT=w_t[:, ts(c)], rhs=ident16[:],
                                     is_transpose=True, start=(c == 0), stop=(c == 3))
                wT = p_w.tile([P, D], F16, name="wT", tag="wT")
                nc.scalar.copy(out=wT[:], in_=tpw[:])

                # 14. gate = sigmoid(w @ Wg + bg) = exp(-ln(1+exp(-g)))
                ps_g = ps_mm.tile([P, D], F32, name="mm", tag="mm")
                for c in range(4):
                    nc.tensor.matmul(ps_g[:], lhsT=wT[:, ts(c)], rhs=wg_sb[:, c, :],
                                     start=(c == 0), stop=False)
                nc.tensor.matmul(ps_g[:], lhsT=ones16[0:1, :], rhs=bg_sb[0:1, :],
                                 start=False, stop=True)
                eg = p_w.tile([P, D], F32, name="eg", tag="eg")
                nc.scalar.activation(out=eg[:], in_=ps_g[:], func=AF.Exp, scale=-1.0)
                nc.scalar.activation(out=eg[:], in_=eg[:], func=AF.Identity, bias=1.0)
                nc.scalar.activation(out=eg[:], in_=eg[:], func=AF.Ln)
                gate = p_w.tile([P, D], F16, name="gate", tag="gate")
                nc.scalar.activation(out=gate[:], in_=eg[:], func=AF.Exp, scale=-1.0)

                # 15. z = mp + gate*w ; out = LN2(z)
                z_t = p_w.tile([P, D], F16, name="z", tag="z")
                nc.vector.tensor_mul(out=z_t[:], in0=gate[:], in1=w_t[:])
                nc.vector.tensor_add(out=z_t[:], in0=z_t[:], in1=x_m[:])
                rstd2, nmr2 = ln_stats(z_t[:])
                out_t = p_w.tile([P, D], F32, name="outt", tag="outt")
                ln_apply(z_t[:], out_t, rstd2, nmr2, 2 if need_aff2 else None)
                nc.sync.dma_start(out=out_ext[r0:r0 + P, :], in_=out_t[:])

            # 3-stage software pipeline: emit A1(i), A2(i-1), B(i-2) each step so
            # every engine FIFO interleaves three tiles; one tile's cross-engine
            # waits hide behind other tiles' queued work.
            def pe_heartbeat(n):
                """Dependency-free filler matmuls that keep the PE HAM window
                busy while real work waits on cross-engine results."""
                dmy = ps_mm.tile([P, 256], F32, name="dmy", tag="dmy", bufs=1)
                for j in range(n):
                    nc.tensor.matmul(dmy[:], lhsT=wproj_sb[:, 0, 0:P],
                                     rhs=wproj_sb[:, 0, 0:256],
                                     start=(j == 0), stop=(j == n - 1))

            from collections import deque
            q1, q2 = deque(), deque()
            for i in range(ntiles):
                q1.append(emitA1(i))
                pe_heartbeat(12)
                if len(q1) > 1:
                    q2.append(emitA2(q1.popleft()))
                if len(q2) > 1:
                    emitB(q2.popleft())
            q2.append(emitA2(q1.popleft()))
            while q2:
                emitB(q2.popleft())
    nc.finalize()
    return nc


def kernel(**inputs):
    global LAST_EXEC_TIME_NS, LAST_RESULTS
    inputs = {k: np.ascontiguousarray(np.asarray(v)) for k, v in inputs.items()}
    Bfull = inputs['frontier'].shape[0]
    assert Bfull % (NCORES * P) == 0
    R = Bfull // NCORES

    folded, aff = _fold(inputs)
    bxo = np.concatenate([inputs["bf"] + inputs["bo"], inputs["br"] + inputs["bo"]])[None, :].astype(np.float16)
    need_aff1 = not (np.allclose(aff[0], 1.0) and np.allclose(aff[1], 0.0))
    need_aff2 = not (np.allclose(aff[2], 1.0) and np.allclose(aff[3], 0.0))
    nc = _build(R, need_aff1, need_aff2)

    in_maps = []
    for c in range(NCORES):
        m = {n: inputs[n][c * R:(c + 1) * R] for n in ("frontier", "cross_robot", "map_feat")}
        m.update(folded)
        m["bxo"] = bxo
        if need_aff1 or need_aff2:
            m["aff"] = aff
        in_maps.append(m)

    trace = bool(os.environ.get("KERNEL_TRACE"))
    res = run_bass_kernel_spmd(nc, in_maps, core_ids=list(range(NCORES)), trace=trace)
    LAST_EXEC_TIME_NS = res.exec_time_ns
    LAST_RESULTS = res
    out = np.concatenate([res.results[c]["out"] for c in range(NCORES)], axis=0)
    return out.astype(np.float32)

